# revision 6
# baseline (speedup 1.0000x reference)
"""CondConv (per-sample dynamic conv) Trainium2 Bass kernel.

Reference computation (per sample b):
    gap     = mean(x[b], spatial)                    # [C]
    r       = sigmoid(fc_w @ gap + fc_b)             # [E]
    comb    = sum_e r[e] * kernel_weights[e]         # [O, I, 3, 3]
    y[b]    = conv2d(x[b], comb, pad=1)              # [O, H, W]

Sharding: data-parallel over batch, 4 samples per core on 8 cores.
Expert kernels + fc params replicated to every core.

Per-core dataflow (v3 — startup/tail optimized):
  - W host layout is [oh][ci][tap][e][oin] (tap-major) so the first
    synthesis tap-group's weights (taps 0-2, all experts) are the first
    contiguous W bytes off the wire.
  - Sample 0's x rides both HWDGE rings as 8 quarter-chunks (ci0 on
    sync -> ACT cast+GAP-accum; ci1 on scalar -> DVE copy + reduce), so
    the GAP tail after the last chunk is ~1us.  Routing accumulates 8
    rank-1 matmuls; sigmoid's ACT table is preloaded by a dummy op.
  - Broadcast of r to 128 partitions via a bf16 eye-matmul.
  - Warm-up matmul batches bracket the routing so the PE HAM clock-gate
    is 8/8 when the conv stream starts and never re-throttles.
  - Synthesis of the first (oh0,ci0) chunk is split into 3 tap-groups so
    the conv stream starts ~3us after routing instead of ~8us.
  - Conv per (s,oh) block: tap-outer / tile-inner over 7 PSUM banks.
    The last two blocks run tile-outer (18 accumulating matmuls per
    PSUM tile, evacuate+store per tile) so the y stores spread across
    the block instead of bunching after the final matmul.
  - Steady-state staging/casts on ACT, x(s+1) DMA triggers emitted
    before conv block (s,0) so they sit ahead of that block's store
    triggers in the ring FIFOs.
"""

import numpy as np
import ml_dtypes

B, C, H, W = 32, 256, 56, 56
E = 8
N_CORES = 8
BL = B // N_CORES          # local batch per core
HP = 58                    # padded rows (1 top + 1 bottom)
WP = 60                    # padded cols (2 left + 2 right: keeps the
                           # bf16 interior 4B-aligned for DVE/ACT 2x)
HWP = HP * WP              # 3480
HWU = H * W                # 3136 (unpadded)
HH = H // 2                # 28 rows per steady staging half-chunk
QH = H // 4                # 14 rows per hot-sample quarter-chunk
TAPS = 9
OIN = 128                  # output channels per half
CIBLK = TAPS * OIN         # per (oh, ci) combined block = 1152
OHBLK = 2 * CIBLK          # per (oh) block = 2304
EBLK = 2 * OHBLK           # combined weights per sample = 4608
WBLK = TAPS * E * OIN      # per (oh, ci) expert-stack block = 9216
ROWS = 8                   # output rows per n-tile
NT = H // ROWS             # 7 n-tiles
NF = ROWS * W              # 448 matmul free dim

_CACHE = {}


def _build():
    import concourse.bacc as bacc
    import concourse.mybir as mybir
    import concourse.tile as tile
    from contextlib import ExitStack

    dt = mybir.dt
    AF = mybir.ActivationFunctionType
    Alu = mybir.AluOpType
    AX = mybir.AxisListType

    nc = bacc.Bacc(
        "TRN2",
        target_bir_lowering=False,
        debug=False,
        enable_asserts=False,
        num_devices=N_CORES,
    )
    x_d = nc.dram_tensor("x", [BL, C, H, W], dt.float32, kind="ExternalInput")
    # host layout per partition p (= i % 128): [oh, ci, tap, e, oin]
    w_d = nc.dram_tensor("wp", [128, 4 * WBLK], dt.bfloat16, kind="ExternalInput")
    fcw_d = nc.dram_tensor("fcw", [C, E], dt.float32, kind="ExternalInput")
    fcb_d = nc.dram_tensor("fcb", [E, 1], dt.float32, kind="ExternalInput")
    eye_d = nc.dram_tensor("eye", [E, E], dt.bfloat16, kind="ExternalInput")
    y_d = nc.dram_tensor("y", [BL, C, H, W], dt.float32, kind="ExternalOutput")

    with tile.TileContext(nc) as tc:
        with ExitStack() as ctx:
            cpool = ctx.enter_context(tc.tile_pool(name="consts", bufs=1))
            xpool = ctx.enter_context(tc.tile_pool(name="xs", bufs=3))
            combpool = ctx.enter_context(tc.tile_pool(name="combs", bufs=2))
            spool = ctx.enter_context(tc.tile_pool(name="small", bufs=2))
            opool = ctx.enter_context(tc.tile_pool(name="outs", bufs=2))
            pspool = ctx.enter_context(tc.tile_pool(name="cpsum", bufs=1, space="PSUM"))
            psmall = ctx.enter_context(tc.tile_pool(name="spsum", bufs=1, space="PSUM"))

            w_sb = cpool.tile([128, 4 * WBLK], dt.bfloat16)
            wv = w_sb.rearrange(
                "p (oh ci tap e o) -> p oh ci tap e o", oh=2, ci=2, tap=TAPS, e=E, o=OIN
            )
            fcw_sb = cpool.tile([128, 2 * E], dt.float32)
            fcb_sb = cpool.tile([E, 1], dt.float32)
            eye_sb = cpool.tile([E, E], dt.bfloat16)
            # persistent fp32 staging buffers, one per ci-half; chunk DMAs
            # and casts address slices so Tile's range-level deps pipeline
            stg = [
                cpool.tile([128, HWU], dt.float32, name=f"stg{ci}") for ci in range(2)
            ]
            dum_in = cpool.tile([1, 1], dt.float32)
            dum_out = cpool.tile([1, 1], dt.bfloat16)
            warm = cpool.tile([128, 576], dt.bfloat16)

            xvs, gaps, rbs, combs = {}, {}, {}, {}

            # ---- DMA emission helpers (explicit ring assignment) ----
            def dma_w(eng, oh, ci, t0, t1):
                base = (oh * 2 + ci) * WBLK
                lo = base + t0 * E * OIN
                hi = base + t1 * E * OIN
                eng.dma_start(out=w_sb[:, lo:hi], in_=w_d.ap()[:, lo:hi])

            def dma_consts_fcw():
                for ci in range(2):
                    nc.scalar.dma_start(
                        out=fcw_sb[:, ci * E : (ci + 1) * E],
                        in_=fcw_d.ap()[ci * 128 : (ci + 1) * 128, :],
                    )
                nc.scalar.dma_start(out=fcb_sb[:], in_=fcb_d.ap())
                nc.scalar.dma_start(out=eye_sb[:], in_=eye_d.ap())

            # ---- staging: x DMA + cast to padded bf16 + GAP ----
            def dma_x_chunk(eng, s, ci, h0, h1):
                eng.dma_start(
                    out=stg[ci][:, h0 * W : h1 * W],
                    in_=x_d.ap()[s, ci * 128 : (ci + 1) * 128, h0:h1, :],
                )

            xts = {}

            def make_xt(s):
                xt = xpool.tile([128, 2 * HWP], dt.bfloat16, tag="xt")
                xv = xt.rearrange("p (c h w) -> p c h w", c=2, h=HP, w=WP)
                xvs[s] = xv
                xts[s] = xt
                for ci in range(2):
                    nc.gpsimd.memset(xv[:, ci, 0, :], 0.0)
                    nc.gpsimd.memset(xv[:, ci, HP - 1, :], 0.0)
                    nc.gpsimd.memset(xv[:, ci, 1 : HP - 1, 0:2], 0.0)
                    nc.gpsimd.memset(xv[:, ci, 1 : HP - 1, WP - 2 : WP], 0.0)
                g = spool.tile([128, 8], dt.float32, tag="gap")
                gaps[s] = g
                return xv, g

            def cast_chunk_act(s, ci, h0, h1, gcol):
                # fp32 -> bf16 into the padded layout AND spatial-sum for GAP
                xv = xvs[s]
                xgv = stg[ci].rearrange("p (h w) -> p h w", h=H, w=W)
                nc.scalar.activation(
                    out=xv[:, ci, 1 + h0 : 1 + h1, 2 : 2 + W],
                    in_=xgv[:, h0:h1, :],
                    func=AF.Copy,
                    accum_out=gaps[s][:, gcol : gcol + 1],
                )

            def cast_chunk_dve(s, ci, h0, h1, gcol):
                xv = xvs[s]
                xgv = stg[ci].rearrange("p (h w) -> p h w", h=H, w=W)
                nc.vector.tensor_copy(
                    out=xv[:, ci, 1 + h0 : 1 + h1, 2 : 2 + W], in_=xgv[:, h0:h1, :]
                )
                nc.vector.reduce_sum(
                    gaps[s][:, gcol : gcol + 1],
                    stg[ci][:, h0 * W : h1 * W],
                    axis=AX.X,
                )

            def stage_dma(s):
                dma_x_chunk(nc.sync, s, 0, 0, HH)
                dma_x_chunk(nc.sync, s, 0, HH, H)
                dma_x_chunk(nc.scalar, s, 1, 0, HH)
                dma_x_chunk(nc.scalar, s, 1, HH, H)

            def stage_cast(s):
                cast_chunk_act(s, 0, 0, HH, 0)
                cast_chunk_act(s, 0, HH, H, 1)
                cast_chunk_act(s, 1, 0, HH, 2)
                cast_chunk_act(s, 1, HH, H, 3)

            # ---- routing: logits -> sigmoid -> broadcast to 128p ----
            # gap cols [0, n0) hold ci0 partial sums, [n0, n0+n1) ci1; two
            # matmuls land partials in disjoint PSUM columns, a tiny DVE
            # row-reduce folds them into the logit vector.
            def route(s, n0, n1):
                g = gaps[s]
                prt = psmall.tile([128, E], dt.float32, tag="prt")
                nc.tensor.matmul(
                    prt[0:E, 0:n0],
                    lhsT=fcw_sb[:, 0:E],
                    rhs=g[:, 0:n0],
                    start=True,
                    stop=True,
                )
                nc.tensor.matmul(
                    prt[0:E, n0 : n0 + n1],
                    lhsT=fcw_sb[:, E : 2 * E],
                    rhs=g[:, n0 : n0 + n1],
                    start=True,
                    stop=True,
                )
                lg = spool.tile([E, 1], dt.float32, tag="lg")
                nc.vector.reduce_sum(lg[:], prt[0:E, 0 : n0 + n1], axis=AX.X)
                rr = spool.tile([E, 1], dt.bfloat16, tag="rr")
                nc.scalar.activation(
                    out=rr[:], in_=lg[:], func=AF.Sigmoid, bias=fcb_sb[:], scale=1.0
                )
                # broadcast r to all 128 partitions via bf16 eye-matmul
                nc.tensor.matmul(
                    prt[:, 0:E],
                    lhsT=rr[:].broadcast_to([E, 128]),
                    rhs=eye_sb[:],
                    start=True,
                    stop=True,
                )
                rb = spool.tile([128, E], dt.float32, tag="rb")
                nc.scalar.activation(out=rb[:], in_=prt[:, 0:E], func=AF.Copy)
                rbs[s] = rb

            # ---- synthesis of one (oh, ci) chunk, by tap-groups ----
            # Each group [t0, t1): DVE chain of tensor_scalar (scale) +
            # tensor_tensor (accumulate) over the expert stack; experts in
            # act_experts have their scale offloaded to ScalarE.
            def synth_chunk(s, oh, ci, groups=((0, TAPS),), act_experts=()):
                cb = combs[s]
                cbv = cb.rearrange(
                    "p (oh ci tap o) -> p oh ci tap o", oh=2, ci=2, tap=TAPS, o=OIN
                )
                rb = rbs[s]
                for t0, t1 in groups:
                    nt_ = t1 - t0
                    fd = nt_ * OIN
                    dst = cbv[:, oh, ci, t0:t1, :]
                    atmps = {}
                    for e in act_experts:
                        at = spool.tile(
                            [128, fd], dt.bfloat16, tag=f"atmp{e}", name=f"atmp{e}"
                        )
                        atv = at.rearrange("p (t o) -> p t o", t=nt_, o=OIN)
                        nc.scalar.activation(
                            out=atv[:],
                            in_=wv[:, oh, ci, t0:t1, e, :],
                            func=AF.Copy,
                            scale=rb[:, e : e + 1],
                        )
                        atmps[e] = atv
                    first = True
                    for e in range(E):
                        src = wv[:, oh, ci, t0:t1, e, :]
                        if e in atmps:
                            nc.vector.tensor_tensor(
                                out=dst, in0=atmps[e][:], in1=dst, op=Alu.add
                            )
                        elif first:
                            nc.vector.tensor_scalar_mul(dst, src, rb[:, e : e + 1])
                            first = False
                        else:
                            tmp = spool.tile(
                                [128, fd], dt.bfloat16, tag="stmp", name="stmp"
                            )
                            tmpv = tmp.rearrange("p (t o) -> p t o", t=nt_, o=OIN)
                            nc.vector.tensor_scalar_mul(tmpv[:], src, rb[:, e : e + 1])
                            nc.vector.tensor_tensor(
                                out=dst, in0=tmpv[:], in1=dst, op=Alu.add
                            )

            def new_cb(s):
                cb = combpool.tile([128, EBLK], dt.bfloat16, tag="cb")
                combs[s] = cb

            # ---- conv of one (s, oh) block ----
            # tap-outer (default): each (ci,tap) lhsT streams 7 matmuls, all
            # 7 PSUM tiles accumulate in parallel; quarter-stores ride both
            # rings as rows complete.
            # tile-outer (tail blocks): 18 accumulating matmuls per PSUM
            # tile, evacuate + store per tile so stores spread evenly.
            def conv_block(s, oh, tile_outer=False, last=False):
                xv = xvs[s]
                cb = combs[s]
                ot = opool.tile([128, HWU], dt.float32, tag="ot")
                otv = ot.rearrange("p (h w) -> p h w", h=H, w=W)
                pss = [
                    pspool.tile([128, NF], dt.float32, tag=f"ps{nt}", name=f"ps{nt}")
                    for nt in range(NT)
                ]

                def tap_iter():
                    for ci in range(2):
                        for kh in range(3):
                            for kw in range(3):
                                lo = (oh * 2 + ci) * CIBLK + (kh * 3 + kw) * OIN
                                yield ci, kh, kw, cb[:, lo : lo + OIN]

                if tile_outer:
                    for nt in range(NT):
                        r0 = nt * ROWS
                        for k, (ci, kh, kw, lhsT) in enumerate(tap_iter()):
                            nc.tensor.matmul(
                                pss[nt],
                                lhsT=lhsT,
                                rhs=xv[
                                    :,
                                    ci,
                                    r0 + kh : r0 + kh + ROWS,
                                    kw + 1 : kw + 1 + W,
                                ],
                                start=(k == 0),
                                stop=(k == 17),
                            )
                        dstc = ot[:, nt * NF : (nt + 1) * NF]
                        if nt % 2 == 1:
                            nc.vector.tensor_copy(out=dstc, in_=pss[nt][:])
                        else:
                            nc.scalar.activation(out=dstc, in_=pss[nt][:], func=AF.Copy)
                        qa, qb = nt * ROWS, (nt + 1) * ROWS
                        if last and nt == NT - 1:
                            # split the final store across both rings
                            nc.sync.dma_start(
                                out=y_d.ap()[s, oh * 128 : (oh + 1) * 128, qa : qa + 4, :],
                                in_=otv[:, qa : qa + 4, :],
                            )
                            nc.scalar.dma_start(
                                out=y_d.ap()[s, oh * 128 : (oh + 1) * 128, qa + 4 : qb, :],
                                in_=otv[:, qa + 4 : qb, :],
                            )
                        else:
                            eng = nc.sync if nt % 2 == 0 else nc.scalar
                            eng.dma_start(
                                out=y_d.ap()[s, oh * 128 : (oh + 1) * 128, qa:qb, :],
                                in_=otv[:, qa:qb, :],
                            )
                    return

                k = 0
                for ci, kh, kw, lhsT in tap_iter():
                    for nt in range(NT):
                        r0 = nt * ROWS
                        nc.tensor.matmul(
                            pss[nt],
                            lhsT=lhsT,
                            rhs=xv[
                                :,
                                ci,
                                r0 + kh : r0 + kh + ROWS,
                                kw + 1 : kw + 1 + W,
                            ],
                            start=(k == 0),
                            stop=(k == 17),
                        )
                    k += 1
                # evacuate PSUM -> SBUF; store as rows complete, spreading
                # the stores across both HWDGE rings.
                stores = {1: (0, 14), 3: (14, 28), 5: (28, 42), 6: (42, 56)}
                half = 0
                for nt in range(NT):
                    dstc = ot[:, nt * NF : (nt + 1) * NF]
                    nc.scalar.activation(out=dstc, in_=pss[nt][:], func=AF.Copy)
                    if nt in stores:
                        qa, qb = stores[nt]
                        eng = nc.sync if half == 0 else nc.scalar
                        half ^= 1
                        eng.dma_start(
                            out=y_d.ap()[s, oh * 128 : (oh + 1) * 128, qa:qb, :],
                            in_=otv[:, qa:qb, :],
                        )

            # ================= emission =================
            # Ring plan (startup):
            #   sync HWDGE:   x-s0 ci0 quarters | W(0,0) taps0-3 | W(0,0)
            #                 taps3-9 | x-s1 ci0 | W(1,0) | stores ...
            #   scalar HWDGE: x-s0 ci1 q0 | fcw,fcb,eye | ci1 q1-3 | W(0,1)
            #                 | x-s1 ci1 | stores ...
            #   gpsimd SWDGE: W(1,1) (slow ring, needed last)
            nc.gpsimd.memset(dum_in[:], 0.0)
            nc.vector.memset(warm[:], 0.0)
            xv0, g0 = make_xt(0)
            # preload the sigmoid ACT table off the critical path
            nc.scalar.activation(
                out=dum_out[:], in_=dum_in[:], func=AF.Sigmoid, scale=1.0
            )

            for q in range(4):
                dma_x_chunk(nc.sync, 0, 0, q * QH, (q + 1) * QH)
                if q == 0:
                    dma_x_chunk(nc.scalar, 0, 1, 0, QH)
                    dma_consts_fcw()
                else:
                    dma_x_chunk(nc.scalar, 0, 1, q * QH, (q + 1) * QH)
            dma_w(nc.gpsimd, 1, 1, 0, TAPS)
            dma_w(nc.sync, 0, 0, 0, 3)
            dma_w(nc.sync, 0, 0, 3, TAPS)
            dma_w(nc.scalar, 0, 1, 0, TAPS)

            for q in range(4):
                cast_chunk_act(0, 0, q * QH, (q + 1) * QH, q)
            for q in range(4):
                cast_chunk_dve(0, 1, q * QH, (q + 1) * QH, 4 + q)

            def warm_mms(n, tag="ps0"):
                psw = pspool.tile([128, NF], dt.float32, tag=tag, name="psw")
                for i in range(n):
                    nc.tensor.matmul(
                        psw[:], lhsT=warm[:, 448:576], rhs=warm[:, 0:448],
                        start=(i == 0), stop=(i == n - 1),
                    )

            warm_mms(30)
            route(0, 4, 4)
            warm_mms(6, tag="ps1")

            new_cb(0)
            s0_act = (5, 6, 7)
            synth_chunk(0, 0, 0, groups=((0, 3), (3, 6), (6, TAPS)), act_experts=s0_act)
            synth_chunk(0, 0, 1, groups=((0, 5), (5, TAPS)), act_experts=s0_act)
            xv1, g1 = make_xt(1)
            stage_dma(1)
            stage_cast(1)
            dma_w(nc.sync, 1, 0, 0, TAPS)
            conv_block(0, 0)

            route(1, 2, 2)
            synth_chunk(0, 1, 0, act_experts=s0_act)
            synth_chunk(0, 1, 1, act_experts=s0_act)
            conv_block(0, 1)

            new_cb(1)
            synth_chunk(1, 0, 0, act_experts=s0_act)
            synth_chunk(1, 0, 1, act_experts=s0_act)
            xv2, g2 = make_xt(2)
            stage_dma(2)
            stage_cast(2)
            conv_block(1, 0)

            route(2, 2, 2)
            synth_chunk(1, 1, 0)
            synth_chunk(1, 1, 1)
            conv_block(1, 1)

            new_cb(2)
            synth_chunk(2, 0, 0)
            synth_chunk(2, 0, 1)
            xv3, g3 = make_xt(3)
            stage_dma(3)
            stage_cast(3)
            conv_block(2, 0)

            route(3, 2, 2)
            synth_chunk(2, 1, 0)
            synth_chunk(2, 1, 1)
            conv_block(2, 1)

            new_cb(3)
            synth_chunk(3, 0, 0, act_experts=s0_act)
            synth_chunk(3, 0, 1, act_experts=s0_act)
            synth_chunk(3, 1, 0, act_experts=s0_act)
            synth_chunk(3, 1, 1, act_experts=s0_act)
            conv_block(3, 0, tile_outer=True)
            conv_block(3, 1, tile_outer=True, last=True)

    nc.compile()
    return nc


def _get_nc():
    if "nc" not in _CACHE:
        _CACHE["nc"] = _build()
    return _CACHE["nc"]


def _pack_inputs(x, kernel_weights, fc_w, fc_b):
    # w layout per partition p (= i % 128): [oh, ci, tap, e, oin], bf16
    a = np.asarray(kernel_weights, np.float32).reshape(E, 2, 128, 2, 128, 3, 3)
    # dims: e, oh, oin, ci, p, kh, kw -> p, oh, ci, (kh kw), e, oin
    a = np.ascontiguousarray(a.transpose(4, 1, 3, 5, 6, 0, 2)).reshape(128, 4 * WBLK)
    wp = a.astype(ml_dtypes.bfloat16)
    fcw_t = np.ascontiguousarray(np.asarray(fc_w, np.float32).T / float(H * W))
    fcb2 = np.ascontiguousarray(np.asarray(fc_b, np.float32).reshape(E, 1))
    eye = np.eye(E, dtype=np.float32).astype(ml_dtypes.bfloat16)
    x = np.ascontiguousarray(np.asarray(x, np.float32))
    in_maps = []
    for i in range(N_CORES):
        in_maps.append(
            {
                "x": x[i * BL : (i + 1) * BL],
                "wp": wp,
                "fcw": fcw_t,
                "fcb": fcb2,
                "eye": eye,
            }
        )
    return in_maps


def _run(x, kernel_weights, fc_w, fc_b, trace=False):
    from concourse.bass_utils import run_bass_kernel_spmd

    nc = _get_nc()
    in_maps = _pack_inputs(x, kernel_weights, fc_w, fc_b)
    res = run_bass_kernel_spmd(nc, in_maps, core_ids=list(range(N_CORES)), trace=trace)
    y = np.concatenate([res.results[i]["y"] for i in range(N_CORES)], axis=0)
    return np.ascontiguousarray(y.astype(np.float32)), res


def kernel(x, kernel_weights, fc_w, fc_b):
    y, _ = _run(x, kernel_weights, fc_w, fc_b, trace=False)
    return y


def kernel_traced(x, kernel_weights, fc_w, fc_b):
    y, res = _run(x, kernel_weights, fc_w, fc_b, trace=True)
    return y, res
